# revision 12
# baseline (speedup 1.0000x reference)
"""Trainium2 Bass kernel for nn_AttentionLayer (B=16, T=2048, D=256), 8 cores.

Math (per batch b):
    h  = input[:, :256] + input[:, 256:512]            # [T, D]
    aw = relu(h @ W.T + b)                             # [T, D]
    m  = tanh(h)
    S  = m @ aw.T                                      # [T, T]
    P  = softmax(S, axis=-1)
    out = h.T + h.T @ P                                # [D, T]

Sharding: data-parallel over batch. 16 batches -> 2 per NeuronCore.

Kernel structure per batch (all bf16 on the TensorEngine, f32 PSUM accum):
    S1 : DMA input row-blocks [128, 512], h_td = half0 + half1 (bf16).
    S1b: transpose h_td via identity-matmul -> hT; mT = tanh(hT) (ScalarE).
    S2 : awT = relu(W.T-matmul + bias) (bias+relu fused on VectorE).
    S3 : per t-block: S = mT.T @ awT (PSUM), E = exp(S - 45) on ScalarE;
         row-sum of E on VectorE (bf16 SBUF 4x mode); g = h_td / rowsum.
         The constant -45 shift is mathematically exact for softmax (cancels
         in the normalization) and keeps exp() in range without a max pass.
    S4 : out[dh, sc] = sum_t g[t, d] * E[t, s] accumulated in PSUM over the
         16 t-blocks; residual h.T added during the PSUM->SBUF evacuation.

Software pipeline (b_loc == 2):
    warmup-MMs | S1(0) S1b(0) S2(0) | S1(1); S3(0)+[S1b(1) S2(1) blocks]
    | S3(1)+[S4(0) regions] | S4(1)
The warmup matmuls keep TensorE busy from t~1us so the HAM clock-gate
reaches 2.4 GHz before the real matmul stream begins.  S4(b-1) regions
interleave into S3(b) so TensorE's S4 stream absorbs ScalarE's exp()
latency.  PSUM: 'ps' pool (2 x [128,1024] = 4 banks) for S1b/S2/S3,
'pso' pool (4 banks) for S4 accumulators.
"""

import numpy as np

import concourse.bass as bass
import concourse.mybir as mybir
import concourse.tile as tile
from concourse import bacc
from concourse.bass_utils import run_bass_kernel_spmd
from concourse.masks import make_identity

N_CORES = 8
EXP_SHIFT = -45.0  # exact for softmax; bounds exp() inputs
N_WARMUP_MM = 40


def build_kernel(nc, b_loc: int, t: int, d: int):
    """Emit the Tile program. t = seq len, d = feature dim (256)."""
    f32 = mybir.dt.float32
    bf16 = mybir.dt.bfloat16
    P = 128
    MMN = min(512, t)     # moving-operand width per matmul
    SC = min(1024, t)     # psum tile / ACT-op width
    ntb = t // P          # t-blocks per batch
    d_halves = d // P     # 2
    nsc = t // SC         # s-chunks per row

    inp = nc.dram_tensor("input_feature", [b_loc, t, 2 * d], f32,
                         kind="ExternalInput").ap()
    W = nc.dram_tensor("W", [d, d], f32, kind="ExternalInput").ap()
    bias = nc.dram_tensor("b", [d], f32, kind="ExternalInput").ap()
    out = nc.dram_tensor("out", [b_loc, d, t], f32,
                         kind="ExternalOutput").ap()

    with tile.TileContext(nc) as tc:
        with (
            tc.tile_pool(name="const", bufs=1) as const,
            tc.tile_pool(name="ps", bufs=2, space="PSUM") as ps,
            tc.tile_pool(name="pso", bufs=2, space="PSUM") as pso,
            tc.tile_pool(name="inp_p", bufs=4) as inp_p,
            tc.tile_pool(name="h_p", bufs=2 * ntb) as h_p,
            tc.tile_pool(name="g_p", bufs=2 * ntb) as g_p,
            tc.tile_pool(name="hT_p", bufs=2 * d_halves) as hT_p,
            tc.tile_pool(name="mT_p", bufs=2 * d_halves) as mT_p,
            tc.tile_pool(name="awT_p", bufs=2 * d_halves) as awT_p,
            tc.tile_pool(name="E_p", bufs=min(2 * ntb, ntb + 8)) as E_p,
            tc.tile_pool(name="z_p", bufs=8) as z_p,
            tc.tile_pool(name="out_p", bufs=3) as out_p,
        ):
            # ---- setup: identity, W^T (bf16), bias ----
            ident = const.tile([P, P], bf16)
            make_identity(nc, ident[:])

            b_sb = const.tile([P, d_halves], f32)
            nc.sync.dma_start(out=b_sb[:], in_=bias.rearrange("(h p) -> p h", p=P))

            shift = const.tile([P, 1], f32)
            nc.vector.memset(shift[:], EXP_SHIFT)

            w_stage = []
            for k in range(d_halves):
                wf = const.tile([P, d], f32, tag=f"w_f32_{k}")
                nc.sync.dma_start(out=wf[:], in_=W[k * P:(k + 1) * P, :])
                wb = const.tile([P, d], bf16, tag=f"w_bf16_{k}")
                nc.vector.tensor_copy(wb[:], wf[:])
                w_stage.append(wb)

            WT = []
            for dh in range(d_halves):
                ps_w = ps.tile([P, SC], f32, tag="ps", name=f"ps_w{dh}")
                if dh == 0:
                    # HAM warmup: keep TensorE busy while input DMAs stream
                    # so the clock-gate reaches 2.4 GHz before real work.
                    for _ in range(N_WARMUP_MM):
                        nc.tensor.matmul(ps_w[:, 0:P], ident[:], ident[:],
                                         start=True, stop=True)
                for k in range(d_halves):
                    nc.tensor.matmul(ps_w[:, k * P:(k + 1) * P],
                                     w_stage[k][:, dh * P:(dh + 1) * P],
                                     ident[:], start=True, stop=True)
                wt = const.tile([P, d], bf16, tag=f"wt_{dh}")
                nc.vector.tensor_copy(wt[:], ps_w[:, 0:d])
                WT.append(wt)

            # ---------- per-batch stage emitters ----------
            def emit_s1(b):
                h_td = []
                for tb in range(ntb):
                    it = inp_p.tile([P, 2 * d], f32, tag="in", name=f"in{b}_{tb}")
                    nc.sync.dma_start(out=it[:], in_=inp[b, tb * P:(tb + 1) * P, :])
                    ht = h_p.tile([P, d], bf16, tag="h", name=f"h{b}_{tb}")
                    nc.vector.tensor_add(ht[:], it[:, 0:d], it[:, d:2 * d])
                    h_td.append(ht)
                return h_td

            def make_s1b(b, h_td):
                hT = [hT_p.tile([P, t], bf16, tag="hT", name=f"hT{b}_{i}")
                      for i in range(d_halves)]
                mT = [mT_p.tile([P, t], bf16, tag="mT", name=f"mT{b}_{i}")
                      for i in range(d_halves)]

                def block(dh, q):
                    ps_t = ps.tile([P, SC], f32, tag="ps", name=f"ps_t{b}_{dh}_{q}")
                    for j in range(SC // P):
                        tb = q * (SC // P) + j
                        nc.tensor.matmul(ps_t[:, j * P:(j + 1) * P],
                                         h_td[tb][:, dh * P:(dh + 1) * P],
                                         ident[:], start=True, stop=True)
                    sl = slice(q * SC, (q + 1) * SC)
                    nc.scalar.activation(mT[dh][:, sl], ps_t[:],
                                         mybir.ActivationFunctionType.Tanh)
                    nc.vector.tensor_copy(hT[dh][:, sl], ps_t[:])

                blocks = [(lambda dh=dh, q=q: block(dh, q))
                          for dh in range(d_halves) for q in range(nsc)]
                return hT, mT, blocks

            def make_s2(b, hT):
                awT = [awT_p.tile([P, t], bf16, tag="awT", name=f"awT{b}_{i}")
                       for i in range(d_halves)]

                def block(eh, sc):
                    ps_aw = ps.tile([P, SC], f32, tag="ps",
                                    name=f"ps_aw{b}_{eh}_{sc}")
                    for n0 in range(0, SC, MMN):
                        for k in range(d_halves):
                            nc.tensor.matmul(
                                ps_aw[:, n0:n0 + MMN],
                                WT[k][:, eh * P:(eh + 1) * P],
                                hT[k][:, sc * SC + n0:sc * SC + n0 + MMN],
                                start=(k == 0), stop=(k == d_halves - 1))
                    nc.vector.tensor_scalar(
                        out=awT[eh][:, sc * SC:(sc + 1) * SC], in0=ps_aw[:],
                        scalar1=b_sb[:, eh:eh + 1], scalar2=0.0,
                        op0=mybir.AluOpType.add, op1=mybir.AluOpType.max)

                blocks = [(lambda eh=eh, sc=sc: block(eh, sc))
                          for eh in range(d_halves) for sc in range(nsc)]
                return awT, blocks

            def emit_s3(b, h_td, hT, mT, awT, interleave):
                """S = m@aw.T, E = exp(S+shift), g = h/Z.  Pops one closure
                from `interleave` after each t-block."""
                E = [E_p.tile([P, t], bf16, tag="E", name=f"E{b}_{i}")
                     for i in range(ntb)]
                g = [g_p.tile([P, d], bf16, tag="g", name=f"g{b}_{i}")
                     for i in range(ntb)]
                for tb in range(ntb):
                    tsl = slice(tb * P, (tb + 1) * P)
                    zp = z_p.tile([P, nsc], f32, tag="zp")
                    for sc in range(nsc):
                        ps_s = ps.tile([P, SC], f32, tag="ps",
                                       name=f"ps_s{b}_{tb}_{sc}")
                        for n0 in range(0, SC, MMN):
                            for k in range(d_halves):
                                nc.tensor.matmul(
                                    ps_s[:, n0:n0 + MMN], mT[k][:, tsl],
                                    awT[k][:, sc * SC + n0:sc * SC + n0 + MMN],
                                    start=(k == 0), stop=(k == d_halves - 1))
                        nc.scalar.activation(E[tb][:, sc * SC:(sc + 1) * SC],
                                             ps_s[:],
                                             mybir.ActivationFunctionType.Exp,
                                             bias=shift[:], scale=1.0)
                        nc.vector.reduce_sum(zp[:, sc:sc + 1],
                                             E[tb][:, sc * SC:(sc + 1) * SC],
                                             axis=mybir.AxisListType.X)
                    if nsc == 1:
                        zs = zp
                    else:
                        zs = z_p.tile([P, 1], f32, tag="zs")
                        nc.vector.tensor_add(zs[:], zp[:, 0:1], zp[:, 1:2])
                    rinv = z_p.tile([P, 1], f32, tag="rinv")
                    nc.vector.reciprocal(rinv[:], zs[:])
                    nc.vector.tensor_scalar_mul(g[tb][:], h_td[tb][:], rinv[:])
                    if interleave:
                        interleave.pop(0)()
                while interleave:
                    interleave.pop(0)()
                return E, g

            def make_s4(b, E, g, hT):
                def region(dh, sc):
                    ps_o = pso.tile([P, SC], f32, tag="pso",
                                    name=f"pso{b}_{dh}_{sc}")
                    for n0 in range(sc * SC, (sc + 1) * SC, MMN):
                        for tb in range(ntb):
                            nc.tensor.matmul(
                                ps_o[:, n0 - sc * SC:n0 - sc * SC + MMN],
                                g[tb][:, dh * P:(dh + 1) * P],
                                E[tb][:, n0:n0 + MMN],
                                start=(tb == 0), stop=(tb == ntb - 1))
                    ot = out_p.tile([P, SC], f32, tag="out",
                                    name=f"ot{b}_{dh}_{sc}")
                    nc.vector.tensor_add(ot[:], ps_o[:],
                                         hT[dh][:, sc * SC:(sc + 1) * SC])
                    nc.sync.dma_start(
                        out=out[b, dh * P:(dh + 1) * P, sc * SC:(sc + 1) * SC],
                        in_=ot[:])

                return [(lambda dh=dh, sc=sc: region(dh, sc))
                        for dh in range(d_halves) for sc in range(nsc)]

            # ---------- schedule ----------
            if b_loc == 2:
                h0 = emit_s1(0)
                hT0, mT0, blk1b0 = make_s1b(0, h0)
                awT0, blk20 = make_s2(0, hT0)
                for f in blk1b0 + blk20:
                    f()
                h1 = emit_s1(1)
                hT1, mT1, blk1b1 = make_s1b(1, h1)
                awT1, blk21 = make_s2(1, hT1)
                E0, g0 = emit_s3(0, h0, hT0, mT0, awT0, blk1b1 + blk21)
                s40 = make_s4(0, E0, g0, hT0)
                E1, g1 = emit_s3(1, h1, hT1, mT1, awT1, s40)
                for f in make_s4(1, E1, g1, hT1):
                    f()
            else:
                prev_s4 = []
                for b in range(b_loc):
                    h_td = emit_s1(b)
                    hT, mT, blk1b = make_s1b(b, h_td)
                    awT, blk2 = make_s2(b, hT)
                    for f in blk1b + blk2:
                        f()
                    E, g = emit_s3(b, h_td, hT, mT, awT, prev_s4)
                    prev_s4 = make_s4(b, E, g, hT)
                for f in prev_s4:
                    f()
    return nc


_COMPILED = {}


def _get_compiled(b_loc: int, t: int, d: int):
    key = (b_loc, t, d)
    if key not in _COMPILED:
        nc = bacc.Bacc("TRN2", target_bir_lowering=False, debug=False,
                       num_devices=N_CORES)
        build_kernel(nc, b_loc, t, d)
        nc.compile()
        _COMPILED[key] = nc
    return _COMPILED[key]


def kernel(input_feature: np.ndarray, W: np.ndarray, b: np.ndarray,
           trace: bool = False, **extra_kwargs):
    input_feature = np.ascontiguousarray(input_feature, dtype=np.float32)
    W = np.ascontiguousarray(W, dtype=np.float32)
    b = np.ascontiguousarray(b, dtype=np.float32)

    b_full, t, d2 = input_feature.shape
    b_loc = b_full // N_CORES
    nc = _get_compiled(b_loc, t, d2 // 2)

    in_maps = [
        {"input_feature": input_feature[i * b_loc:(i + 1) * b_loc], "W": W, "b": b}
        for i in range(N_CORES)
    ]
    res = run_bass_kernel_spmd(nc, in_maps, core_ids=list(range(N_CORES)),
                               trace=trace, **extra_kwargs)
    out = np.concatenate([r["out"] for r in res.results], axis=0)
    if trace:
        kernel.last_result = res
    return out
